# revision 4
# baseline (speedup 1.0000x reference)
"""TRN2 Bass kernel for nn_MultiHeadAttention (B=2, S=2048, D=1024, H=16).

Sharding: 8 cores = 2 batches x 4 head-groups (4 heads each).
Device math per core (transposed-P layout, f32r matmuls):
  qhT/khT = (Wslice @ xT + b)      [dc, s]   via f32r matmuls, DRAM round-trip
  vh      = (x @ WvT + bv)         [s, dc]   kept in SBUF with ones column
  P^T[k,q] = exp((K Q^T)/8 - 192*mask^T)     mask added in PSUM via fp8 matmul
  ctx^T,s  = [vh|1]^T @ P^T                  AV + rowsum in one matmul
  attn^T   = P^T * (1/s)                     one DVE pass, DMA'd out
  x_part   = (ctx^T/s)^T @ WoT               host sums 4 partials per batch
"""

import sys

sys.path.insert(0, "/opt/trn_rl_repo")

import numpy as np
import ml_dtypes
from contextlib import ExitStack

B, S, D, H = 2, 2048, 1024, 16
HD = 64
HPC = 4            # heads per core
DC = HPC * HD      # 256 local channels per core
NCORES = 8
QC = 512           # q-chunk (columns per attention inner tile)
MASKVAL = -192.0   # fp8-exact; exp(0.125*-192) = e^-24 ~ 3.8e-11

_cache = {}


def _round_f32r(a: np.ndarray) -> np.ndarray:
    """Round fp32 -> e8m11 (f32r) with RNE, in IEEE fp32 bit positions."""
    bits = np.ascontiguousarray(a, np.float32).view(np.uint32)
    lsb = (bits >> 12) & 1
    out = (bits + 0x7FF + lsb) & np.uint32(0xFFFFF000)
    return out.view(np.float32).reshape(a.shape)


def _build(s=S):
    """Build + compile the per-core Bass program (same for all cores)."""
    import concourse.tile as tile
    from concourse import bacc, mybir

    F32 = mybir.dt.float32
    F32R = mybir.dt.float32r
    FP8 = mybir.dt.float8e4
    MM = mybir.AluOpType.mult
    EXPF = mybir.ActivationFunctionType.Exp

    nkb = s // 128    # k blocks
    nqc = s // QC     # q chunks
    nsc = s // 128    # s chunks (output rows)
    nnq = s // 512    # projection column chunks
    nkc = D // 128    # contraction chunks

    nc = bacc.Bacc("TRN2", target_bir_lowering=False, debug=False,
                   num_devices=NCORES)

    XQT = nc.dram_tensor("xqt", [D, s], F32R, kind="ExternalInput").ap()
    XKT = nc.dram_tensor("xkt", [D, s], F32R, kind="ExternalInput").ap()
    XVT = nc.dram_tensor("xvt", [D, s], F32R, kind="ExternalInput").ap()
    WQT = nc.dram_tensor("wqt", [D, DC], F32R, kind="ExternalInput").ap()
    WKT = nc.dram_tensor("wkt", [D, DC], F32R, kind="ExternalInput").ap()
    WVT = nc.dram_tensor("wvt", [D, DC], F32R, kind="ExternalInput").ap()
    WOT = nc.dram_tensor("wot", [HD, HPC, D], F32R, kind="ExternalInput").ap()
    MT8 = nc.dram_tensor("mt8", [s, s], FP8, kind="ExternalInput").ap()
    BQ = nc.dram_tensor("bq_r", [1, DC], F32R, kind="ExternalInput").ap()
    BK = nc.dram_tensor("bk_r", [1, DC], F32R, kind="ExternalInput").ap()
    BV = nc.dram_tensor("bv_r", [1, DC], F32R, kind="ExternalInput").ap()
    ONES128 = nc.dram_tensor("ones128", [1, 128], F32R, kind="ExternalInput").ap()
    ONES512 = nc.dram_tensor("ones512", [1, 512], F32R, kind="ExternalInput").ap()
    I8 = nc.dram_tensor("i8", [128, 128], FP8, kind="ExternalInput").ap()

    ATT = nc.dram_tensor("attn_t", [HPC, s, s], F32, kind="ExternalOutput").ap()
    XO = nc.dram_tensor("x_part", [s, D], F32, kind="ExternalOutput").ap()

    with tile.TileContext(nc) as tc, ExitStack() as ctx:
        # ---- long-lived pools ----
        persist = ctx.enter_context(tc.tile_pool(name="persist", bufs=1))
        dram = ctx.enter_context(tc.tile_pool(name="scratch", bufs=1, space="DRAM"))

        mask_sb = persist.tile([128, nkb, s], FP8, tag="mask")
        for kb in range(nkb):
            nc.sync.dma_start(mask_sb[:, kb, :], MT8[kb * 128:(kb + 1) * 128, :])
        i8_sb = persist.tile([128, 128], FP8, tag="i8")
        nc.sync.dma_start(i8_sb[:], I8)
        consts = persist.tile([1, 128 + 512 + 3 * DC], F32R, tag="consts")
        ones128_sb = consts[:, 0:128]
        ones512_sb = consts[:, 128:640]
        bq_sb = consts[:, 640:640 + DC]
        bk_sb = consts[:, 640 + DC:640 + 2 * DC]
        bv_sb = consts[:, 640 + 2 * DC:640 + 3 * DC]
        nc.sync.dma_start(ones128_sb, ONES128)
        nc.sync.dma_start(ones512_sb, ONES512)
        nc.sync.dma_start(bq_sb, BQ)
        nc.sync.dma_start(bk_sb, BK)
        nc.sync.dma_start(bv_sb, BV)
        woT_sb = persist.tile([HD, HPC, D], F32R, tag="wot")
        nc.sync.dma_start(woT_sb[:], WOT)

        # vh + interleaved ones column: [128, sc, (h,65)]
        vh_sb = persist.tile([128, nsc, HPC * 65], F32R, tag="vh")
        ones_view = vh_sb[:, :, :].rearrange(
            "p c (h d) -> p c h d", h=HPC, d=65)[:, :, :, 64:65]
        nc.gpsimd.memset(ones_view.bitcast(F32), 1.0)

        ctxT_sb = persist.tile([HD, HPC, s], F32R, tag="ctxT")

        qhT_d = dram.tile([DC, s], F32R)
        khT_d = dram.tile([DC, s], F32R)

        # ================= Phase 1: projections =================
        with tc.tile_pool(name="p1w", bufs=1) as p1w, \
             tc.tile_pool(name="p1x", bufs=6) as p1x, \
             tc.tile_pool(name="p1st", bufs=4) as p1st, \
             tc.tile_pool(name="p1ps", bufs=1, space="PSUM") as p1ps:
            wq_sb = p1w.tile([128, nkc, DC], F32R, tag="wq")
            nc.sync.dma_start(wq_sb[:], WQT.rearrange("(c p) n -> p c n", p=128))
            wk_sb = p1w.tile([128, nkc, DC], F32R, tag="wk")
            nc.sync.dma_start(wk_sb[:], WKT.rearrange("(c p) n -> p c n", p=128))
            wv_sb = p1w.tile([128, nkc, DC], F32R, tag="wv")
            nc.sync.dma_start(wv_sb[:], WVT.rearrange("(c p) n -> p c n", p=128))
            for nq in range(nnq):
                ncol = slice(nq * 512, (nq + 1) * 512)
                psq = [p1ps.tile([128, 512], F32, tag=f"psq{m}", name=f"psq{m}") for m in range(2)]
                psk = [p1ps.tile([128, 512], F32, tag=f"psk{m}", name=f"psk{m}") for m in range(2)]
                psv = [p1ps.tile([128, 256], F32, tag=f"psv{j}", name=f"psv{j}") for j in range(4)]
                # bias seeds (K=1)
                for m in range(2):
                    mm = slice(m * 128, (m + 1) * 128)
                    nc.tensor.matmul(psq[m][:], bq_sb[0:1, mm], ones512_sb[0:1, :],
                                     start=True, stop=False)
                    nc.tensor.matmul(psk[m][:], bk_sb[0:1, mm], ones512_sb[0:1, :],
                                     start=True, stop=False)
                for j in range(4):
                    nc.tensor.matmul(psv[j][:], ones128_sb[0:1, 0:128],
                                     bv_sb[0:1, 0:256], start=True, stop=False)
                for kc in range(nkc):
                    last = kc == nkc - 1
                    xq_c = p1x.tile([128, 512], F32R, tag="x")
                    nc.sync.dma_start(xq_c[:], XQT[kc * 128:(kc + 1) * 128, ncol])
                    xk_c = p1x.tile([128, 512], F32R, tag="x")
                    nc.sync.dma_start(xk_c[:], XKT[kc * 128:(kc + 1) * 128, ncol])
                    xv_c = p1x.tile([128, 512], F32R, tag="x")
                    nc.sync.dma_start(xv_c[:], XVT[kc * 128:(kc + 1) * 128, ncol])
                    for m in range(2):
                        mm = slice(m * 128, (m + 1) * 128)
                        nc.tensor.matmul(psq[m][:], wq_sb[:, kc, mm], xq_c[:],
                                         start=False, stop=last)
                        nc.tensor.matmul(psk[m][:], wk_sb[:, kc, mm], xk_c[:],
                                         start=False, stop=last)
                    for sl in range(4):
                        nc.tensor.matmul(psv[sl][:],
                                         xv_c[:, sl * 128:(sl + 1) * 128],
                                         wv_sb[:, kc, :],
                                         start=False, stop=last)
                # drain psums
                for m in range(2):
                    stq = p1st.tile([128, 512], F32R, tag="st")
                    nc.scalar.copy(stq[:], psq[m][:])
                    nc.sync.dma_start(qhT_d[m * 128:(m + 1) * 128, ncol], stq[:])
                    stk = p1st.tile([128, 512], F32R, tag="st")
                    nc.scalar.copy(stk[:], psk[m][:])
                    nc.sync.dma_start(khT_d[m * 128:(m + 1) * 128, ncol], stk[:])
                for sl in range(4):
                    sc = nq * 4 + sl
                    dst = vh_sb[:, sc:sc + 1, :].rearrange(
                        "p one (h d) -> p (one h) d", h=HPC, d=65)[:, :, 0:64]
                    src = psv[sl][:].rearrange(
                        "p (h d) -> p h d", h=HPC, d=64)
                    nc.scalar.copy(dst, src)

        # ================= Phase 2: attention =================
        with tc.tile_pool(name="pkh", bufs=2) as pkh, \
             tc.tile_pool(name="pqh", bufs=1) as pqh, \
             tc.tile_pool(name="pP", bufs=5) as pP, \
             tc.tile_pool(name="prow", bufs=3) as prow, \
             tc.tile_pool(name="prep", bufs=2) as prep, \
             tc.tile_pool(name="pxo", bufs=2) as pxo, \
             tc.tile_pool(name="ppl", bufs=3, space="PSUM") as ppl, \
             tc.tile_pool(name="pctx", bufs=2, space="PSUM") as pctx, \
             tc.tile_pool(name="pmisc", bufs=2, space="PSUM") as pmisc:

            for h in range(HPC):
                kh = pkh.tile([HD, s], F32R, tag="kh")
                nc.sync.dma_start(kh[:], khT_d[h * HD:(h + 1) * HD, :])
                for qc in range(nqc):
                    qsl = slice(qc * QC, (qc + 1) * QC)
                    qh = pqh.tile([HD, QC], F32R, tag="qh")
                    nc.sync.dma_start(qh[:], qhT_d[h * HD:(h + 1) * HD, qsl])
                    Ph = [pP.tile([128, 4, QC], F32R, tag="P", name=f"P{i}")
                          for i in range(nkb // 4)]
                    cps = pctx.tile([65, QC], F32, tag="ctx")
                    for kb in range(nkb):
                        pl = ppl.tile([128, QC], F32, tag="pl")
                        nc.tensor.matmul(pl[:], kh[:, kb * 128:(kb + 1) * 128],
                                         qh[:], start=True, stop=False)
                        nc.tensor.matmul(pl[:], i8_sb[:], mask_sb[:, kb, qsl],
                                         start=False, stop=True)
                        pslice = Ph[kb // 4][:, kb % 4, :]
                        nc.scalar.activation(pslice, pl[:], EXPF, scale=0.125)
                        nc.tensor.matmul(cps[:],
                                         vh_sb[:, kb, h * 65:(h + 1) * 65],
                                         pslice, start=(kb == 0),
                                         stop=(kb == nkb - 1))
                    # softmax denominators
                    s_sb = prow.tile([1, QC], F32, tag="row")
                    nc.scalar.copy(s_sb[:], cps[64:65, :])
                    r_sb = prow.tile([1, QC], F32R, tag="row")
                    with nc.allow_low_precision(reason="f32r recip for rep matmul"):
                        nc.vector.reciprocal(r_sb[:], s_sb[:])
                    rep_ps = pmisc.tile([128, QC], F32, tag="rep")
                    nc.tensor.matmul(rep_ps[:], ones128_sb[0:1, :], r_sb[0:1, :],
                                     start=True, stop=True)
                    rep_sb = prep.tile([128, QC], F32, tag="repsb")
                    nc.scalar.copy(rep_sb[:], rep_ps[:])
                    # normalized ctx^T slice (f32r out)
                    with nc.allow_low_precision(reason="f32r ctx for o-proj"):
                        nc.vector.tensor_tensor(ctxT_sb[0:HD, h, qsl],
                                                cps[0:HD, :], rep_sb[0:HD, :], MM)
                    # normalize P in place, write attn^T
                    for kb in range(nkb):
                        pslice = Ph[kb // 4][:, kb % 4, :]
                        with nc.allow_low_precision(reason="attn f32r"):
                            nc.vector.tensor_tensor(pslice, pslice.bitcast(F32),
                                                    rep_sb[:], MM)
                        nc.sync.dma_start(
                            ATT[h, kb * 128:(kb + 1) * 128, qsl],
                            pslice.bitcast(F32))

            # ================= Phase 3: output projection =================
            for sc in range(nsc):
                ssl = slice(sc * 128, (sc + 1) * 128)
                xo = pxo.tile([128, D], F32, tag="xo")
                for nh in range(2):
                    px = ppl.tile([128, QC], F32, tag="pl")
                    for h in range(HPC):
                        nc.tensor.matmul(px[:], ctxT_sb[0:HD, h, ssl],
                                         woT_sb[0:HD, h, nh * 512:(nh + 1) * 512],
                                         start=(h == 0), stop=(h == HPC - 1))
                    nc.scalar.copy(xo[:, nh * 512:(nh + 1) * 512], px[:])
                nc.sync.dma_start(XO[ssl, :], xo[:])

    nc.compile()
    return nc


def _prep_inputs(v, k, q, mask, Wq, bq, Wk, bk, Wv, bv, Wo, bo):
    """Host-side shard prep. Returns per-core input maps."""
    ones128 = np.ones((1, 128), np.float32)
    ones512 = np.ones((1, 512), np.float32)
    i8 = np.eye(128, dtype=np.float32).astype(ml_dtypes.float8_e4m3)

    xT = {}
    mt8 = {}
    for b in range(B):
        xT[("q", b)] = _round_f32r(np.ascontiguousarray(np.asarray(q[b]).T))
        xT[("k", b)] = _round_f32r(np.ascontiguousarray(np.asarray(k[b]).T))
        xT[("v", b)] = _round_f32r(np.ascontiguousarray(np.asarray(v[b]).T))
        mt8[b] = np.ascontiguousarray(
            np.asarray(mask[b, 0], np.float32).T * MASKVAL
        ).astype(ml_dtypes.float8_e4m3)

    in_maps = []
    for c in range(NCORES):
        b, g = c % B, c // B
        cs = slice(g * DC, (g + 1) * DC)
        wot = np.ascontiguousarray(
            np.asarray(Wo)[:, cs].T.reshape(HPC, HD, D).transpose(1, 0, 2))
        in_maps.append({
            "xqt": xT[("q", b)], "xkt": xT[("k", b)], "xvt": xT[("v", b)],
            "wqt": _round_f32r(np.asarray(Wq)[cs, :].T),
            "wkt": _round_f32r(np.asarray(Wk)[cs, :].T),
            "wvt": _round_f32r(np.asarray(Wv)[cs, :].T),
            "wot": _round_f32r(wot),
            "mt8": mt8[b],
            "bq_r": _round_f32r(np.asarray(bq)[None, cs]),
            "bk_r": _round_f32r(np.asarray(bk)[None, cs]),
            "bv_r": _round_f32r(np.asarray(bv)[None, cs]),
            "ones128": ones128, "ones512": ones512, "i8": i8,
        })
    return in_maps


def kernel(v, k, q, mask, Wq, bq, Wk, bk, Wv, bv, Wo, bo, _trace=False):
    from concourse.bass_utils import run_bass_kernel_spmd

    if "nc" not in _cache:
        _cache["nc"] = _build()
    nc = _cache["nc"]

    in_maps = _prep_inputs(v, k, q, mask, Wq, bq, Wk, bk, Wv, bv, Wo, bo)
    kw = {}
    if _trace:
        kw = dict(trace=True)
    res = run_bass_kernel_spmd(nc, in_maps, core_ids=list(range(NCORES)), **kw)
    _cache["last_result"] = res

    x = np.zeros((B, S, D), np.float32)
    attn = np.empty((B, H, S, S), np.float32)
    for c in range(NCORES):
        b, g = c % B, c // B
        out = res.results[c]
        x[b] += out["x_part"]
        at = out["attn_t"]
        for j in range(HPC):
            attn[b, g * HPC + j] = at[j].T
    x += np.asarray(bo, np.float32)[None, None, :]
    return x, attn


# revision 5
# speedup vs baseline: 1.2954x; 1.2954x over previous
"""TRN2 Bass kernel for nn_MultiHeadAttention (B=2, S=2048, D=1024, H=16).

Sharding: 8 cores = 2 batches x 4 head-groups (4 heads each).
Device math per core (transposed-P layout, fp16 matmuls, fp32 accumulation):
  qhT/khT = (Wslice @ xT + b)      [dc, s]   fp16, SBUF-resident
  vh      = (x @ WvT + bv)         [s, dc]   fp16 in SBUF with ones column
  P^T[k,q] = exp((K Q^T)/8 - 192*mask^T)     mask added in PSUM via fp8 matmul
  ctx^T,s  = [vh|1]^T @ P^T                  AV + rowsum in one matmul
  attn^T   = P^T * (1/s)                     DVE 2x pass, fp16 out to HBM
  x_part   = (ctx^T/s)^T @ WoT               host sums 4 partials per batch
"""

import sys

sys.path.insert(0, "/opt/trn_rl_repo")

import numpy as np
import ml_dtypes
from contextlib import ExitStack

B, S, D, H = 2, 2048, 1024, 16
HD = 64
HPC = 4            # heads per core
DC = HPC * HD      # 256 local channels per core
NCORES = 8
QC = 512           # q-chunk (columns per attention inner tile)
MASKVAL = -192.0   # fp8-exact; exp(0.125*-192) = e^-24 ~ 3.8e-11

_cache = {}


def _build(s=S):
    """Build + compile the per-core Bass program (same for all cores)."""
    import concourse.tile as tile
    from concourse import bacc, mybir

    F32 = mybir.dt.float32
    F16 = mybir.dt.float16
    FP8 = mybir.dt.float8e4
    MM = mybir.AluOpType.mult
    EXPF = mybir.ActivationFunctionType.Exp

    nkb = s // 128    # k blocks
    nqc = s // QC     # q chunks
    nsc = s // 128    # s chunks (output rows)
    nnq = s // 512    # projection column chunks
    nkc = D // 128    # contraction chunks

    nc = bacc.Bacc("TRN2", target_bir_lowering=False, debug=False,
                   num_devices=NCORES)

    XQT = nc.dram_tensor("xqt", [D, s], F16, kind="ExternalInput").ap()
    XKT = nc.dram_tensor("xkt", [D, s], F16, kind="ExternalInput").ap()
    XVT = nc.dram_tensor("xvt", [D, s], F16, kind="ExternalInput").ap()
    WQT = nc.dram_tensor("wqt", [D, DC], F16, kind="ExternalInput").ap()
    WKT = nc.dram_tensor("wkt", [D, DC], F16, kind="ExternalInput").ap()
    WVT = nc.dram_tensor("wvt", [D, DC], F16, kind="ExternalInput").ap()
    WOT = nc.dram_tensor("wot", [HD, HPC, D], F16, kind="ExternalInput").ap()
    MT8 = nc.dram_tensor("mt8", [s, s], FP8, kind="ExternalInput").ap()
    BQ = nc.dram_tensor("bq_r", [1, DC], F16, kind="ExternalInput").ap()
    BK = nc.dram_tensor("bk_r", [1, DC], F16, kind="ExternalInput").ap()
    BV = nc.dram_tensor("bv_r", [1, DC], F16, kind="ExternalInput").ap()
    ONES128 = nc.dram_tensor("ones128", [1, 128], F16, kind="ExternalInput").ap()
    ONES512 = nc.dram_tensor("ones512", [1, 512], F16, kind="ExternalInput").ap()
    I8 = nc.dram_tensor("i8", [128, 128], FP8, kind="ExternalInput").ap()

    ATT = nc.dram_tensor("attn_t", [HPC, s, s], F16, kind="ExternalOutput").ap()
    XO = nc.dram_tensor("x_part", [s, D], F32, kind="ExternalOutput").ap()

    with tile.TileContext(nc) as tc, ExitStack() as ctx:
        # ---- long-lived pools ----
        persist = ctx.enter_context(tc.tile_pool(name="persist", bufs=1))

        mask_sb = persist.tile([128, nkb, s], FP8, tag="mask")
        for kb in range(nkb):
            nc.sync.dma_start(mask_sb[:, kb, :], MT8[kb * 128:(kb + 1) * 128, :])
        i8_sb = persist.tile([128, 128], FP8, tag="i8")
        nc.sync.dma_start(i8_sb[:], I8)
        consts = persist.tile([1, 128 + 512 + 3 * DC], F16, tag="consts")
        ones128_sb = consts[:, 0:128]
        ones512_sb = consts[:, 128:640]
        bq_sb = consts[:, 640:640 + DC]
        bk_sb = consts[:, 640 + DC:640 + 2 * DC]
        bv_sb = consts[:, 640 + 2 * DC:640 + 3 * DC]
        nc.sync.dma_start(ones128_sb, ONES128)
        nc.sync.dma_start(ones512_sb, ONES512)
        nc.sync.dma_start(bq_sb, BQ)
        nc.sync.dma_start(bk_sb, BK)
        nc.sync.dma_start(bv_sb, BV)
        woT_sb = persist.tile([HD, HPC, D], F16, tag="wot")
        nc.sync.dma_start(woT_sb[:], WOT)

        # vh + interleaved ones column: [128, sc, (h,65)]
        vh_sb = persist.tile([128, nsc, HPC * 65], F16, tag="vh")
        ones_view = vh_sb[:, :, :].rearrange(
            "p c (h d) -> p c h d", h=HPC, d=65)[:, :, :, 64:65]
        nc.gpsimd.memset(ones_view, 1.0)

        qhT_sb = persist.tile([128, 2, s], F16, tag="qhT")
        khT_sb = persist.tile([128, 2, s], F16, tag="khT")
        ctxT_sb = persist.tile([HD, HPC, s], F16, tag="ctxT")

        # ================= Phase 1: projections =================
        with tc.tile_pool(name="p1w", bufs=1) as p1w, \
             tc.tile_pool(name="p1x", bufs=6) as p1x, \
             tc.tile_pool(name="p1ps", bufs=1, space="PSUM") as p1ps:
            wq_sb = p1w.tile([128, nkc, DC], F16, tag="wq")
            nc.sync.dma_start(wq_sb[:], WQT.rearrange("(c p) n -> p c n", p=128))
            wk_sb = p1w.tile([128, nkc, DC], F16, tag="wk")
            nc.sync.dma_start(wk_sb[:], WKT.rearrange("(c p) n -> p c n", p=128))
            wv_sb = p1w.tile([128, nkc, DC], F16, tag="wv")
            nc.sync.dma_start(wv_sb[:], WVT.rearrange("(c p) n -> p c n", p=128))
            for nq in range(nnq):
                ncol = slice(nq * 512, (nq + 1) * 512)
                psq = [p1ps.tile([128, 512], F32, tag=f"psq{m}", name=f"psq{m}")
                       for m in range(2)]
                psk = [p1ps.tile([128, 512], F32, tag=f"psk{m}", name=f"psk{m}")
                       for m in range(2)]
                psv = [p1ps.tile([128, 256], F32, tag=f"psv{j}", name=f"psv{j}")
                       for j in range(4)]
                # bias seeds (K=1)
                for m in range(2):
                    mm = slice(m * 128, (m + 1) * 128)
                    nc.tensor.matmul(psq[m][:], bq_sb[0:1, mm], ones512_sb[0:1, :],
                                     start=True, stop=False)
                    nc.tensor.matmul(psk[m][:], bk_sb[0:1, mm], ones512_sb[0:1, :],
                                     start=True, stop=False)
                for j in range(4):
                    nc.tensor.matmul(psv[j][:], ones128_sb[0:1, 0:128],
                                     bv_sb[0:1, 0:256], start=True, stop=False)
                for kc in range(nkc):
                    last = kc == nkc - 1
                    xq_c = p1x.tile([128, 512], F16, tag="x")
                    nc.sync.dma_start(xq_c[:], XQT[kc * 128:(kc + 1) * 128, ncol])
                    xk_c = p1x.tile([128, 512], F16, tag="x")
                    nc.sync.dma_start(xk_c[:], XKT[kc * 128:(kc + 1) * 128, ncol])
                    xv_c = p1x.tile([128, 512], F16, tag="x")
                    nc.sync.dma_start(xv_c[:], XVT[kc * 128:(kc + 1) * 128, ncol])
                    for m in range(2):
                        mm = slice(m * 128, (m + 1) * 128)
                        nc.tensor.matmul(psq[m][:], wq_sb[:, kc, mm], xq_c[:],
                                         start=False, stop=last)
                        nc.tensor.matmul(psk[m][:], wk_sb[:, kc, mm], xk_c[:],
                                         start=False, stop=last)
                    for sl in range(4):
                        nc.tensor.matmul(psv[sl][:],
                                         xv_c[:, sl * 128:(sl + 1) * 128],
                                         wv_sb[:, kc, :],
                                         start=False, stop=last)
                # drain psums into fp16 SBUF residents
                for m in range(2):
                    nc.scalar.copy(qhT_sb[:, m, ncol], psq[m][:])
                    nc.scalar.copy(khT_sb[:, m, ncol], psk[m][:])
                for sl in range(4):
                    sc = nq * 4 + sl
                    dst = vh_sb[:, sc:sc + 1, :].rearrange(
                        "p one (h d) -> p (one h) d", h=HPC, d=65)[:, :, 0:64]
                    src = psv[sl][:].rearrange("p (h d) -> p h d", h=HPC, d=64)
                    nc.scalar.copy(dst, src)

        # ================= Phase 2: attention =================
        with tc.tile_pool(name="pP", bufs=6) as pP, \
             tc.tile_pool(name="prow", bufs=3) as prow, \
             tc.tile_pool(name="prep", bufs=2) as prep, \
             tc.tile_pool(name="pxo", bufs=2) as pxo, \
             tc.tile_pool(name="ppl", bufs=4, space="PSUM") as ppl, \
             tc.tile_pool(name="pctx", bufs=2, space="PSUM") as pctx, \
             tc.tile_pool(name="pmisc", bufs=2, space="PSUM") as pmisc:

            for h in range(HPC):
                hp = slice((h % 2) * 64, (h % 2) * 64 + 64)
                hm = h // 2
                for qc in range(nqc):
                    qsl = slice(qc * QC, (qc + 1) * QC)
                    Ph = [pP.tile([128, 4, QC], F16, tag="P", name=f"P{i}")
                          for i in range(nkb // 4)]
                    cps = pctx.tile([65, QC], F32, tag="ctx")
                    for kb in range(nkb):
                        pl = ppl.tile([128, QC], F32, tag="pl")
                        nc.tensor.matmul(pl[:],
                                         khT_sb[hp, hm, kb * 128:(kb + 1) * 128],
                                         qhT_sb[hp, hm, qsl],
                                         start=True, stop=False)
                        nc.tensor.matmul(pl[:], i8_sb[:], mask_sb[:, kb, qsl],
                                         start=False, stop=True)
                        pslice = Ph[kb // 4][:, kb % 4, :]
                        nc.scalar.activation(pslice, pl[:], EXPF, scale=0.125)
                        nc.tensor.matmul(cps[:],
                                         vh_sb[:, kb, h * 65:(h + 1) * 65],
                                         pslice, start=(kb == 0),
                                         stop=(kb == nkb - 1))
                    # softmax denominators
                    s_sb = prow.tile([1, QC], F32, tag="row")
                    nc.scalar.copy(s_sb[:], cps[64:65, :])
                    r_sb = prow.tile([1, QC], F16, tag="row")
                    with nc.allow_low_precision(reason="fp16 recip"):
                        nc.vector.reciprocal(r_sb[:], s_sb[:])
                    rep_ps = pmisc.tile([128, QC], F32, tag="rep")
                    nc.tensor.matmul(rep_ps[:], ones128_sb[0:1, :], r_sb[0:1, :],
                                     start=True, stop=True)
                    rep_sb = prep.tile([128, QC], F16, tag="repsb")
                    nc.scalar.copy(rep_sb[:], rep_ps[:])
                    # normalized ctx^T slice (fp16 out)
                    with nc.allow_low_precision(reason="fp16 ctx"):
                        nc.vector.tensor_tensor(ctxT_sb[0:HD, h, qsl],
                                                cps[0:HD, :], rep_sb[0:HD, :], MM)
                    # normalize P in place (fp16 2x), batched attn^T writes
                    att_t = ATT[h].rearrange("(kb p) q -> p kb q", p=128)
                    for kb in range(nkb):
                        pslice = Ph[kb // 4][:, kb % 4, :]
                        with nc.allow_low_precision(reason="fp16 attn"):
                            nc.vector.tensor_tensor(pslice, pslice,
                                                    rep_sb[:], MM)
                    for i in range(nkb // 4):
                        nc.sync.dma_start(att_t[:, 4 * i:4 * i + 4, qsl], Ph[i][:])

            # ================= Phase 3: output projection =================
            for sc in range(nsc):
                ssl = slice(sc * 128, (sc + 1) * 128)
                xo = pxo.tile([128, D], F32, tag="xo")
                for nh in range(2):
                    px = ppl.tile([128, QC], F32, tag="pl")
                    for h in range(HPC):
                        nc.tensor.matmul(px[:], ctxT_sb[0:HD, h, ssl],
                                         woT_sb[0:HD, h, nh * 512:(nh + 1) * 512],
                                         start=(h == 0), stop=(h == HPC - 1))
                    nc.scalar.copy(xo[:, nh * 512:(nh + 1) * 512], px[:])
                nc.sync.dma_start(XO[ssl, :], xo[:])

    nc.compile()
    return nc


def _prep_inputs(v, k, q, mask, Wq, bq, Wk, bk, Wv, bv, Wo, bo):
    """Host-side shard prep. Returns per-core input maps."""
    f16 = np.float16
    ones128 = np.ones((1, 128), f16)
    ones512 = np.ones((1, 512), f16)
    i8 = np.eye(128, dtype=np.float32).astype(ml_dtypes.float8_e4m3)

    xT = {}
    mt8 = {}
    for b in range(B):
        xT[("q", b)] = np.ascontiguousarray(np.asarray(q[b]).T).astype(f16)
        xT[("k", b)] = np.ascontiguousarray(np.asarray(k[b]).T).astype(f16)
        xT[("v", b)] = np.ascontiguousarray(np.asarray(v[b]).T).astype(f16)
        mt8[b] = np.ascontiguousarray(
            np.asarray(mask[b, 0], np.float32).T * MASKVAL
        ).astype(ml_dtypes.float8_e4m3)

    in_maps = []
    for c in range(NCORES):
        b, g = c % B, c // B
        cs = slice(g * DC, (g + 1) * DC)
        wot = np.ascontiguousarray(
            np.asarray(Wo)[:, cs].T.reshape(HPC, HD, D).transpose(1, 0, 2))
        in_maps.append({
            "xqt": xT[("q", b)], "xkt": xT[("k", b)], "xvt": xT[("v", b)],
            "wqt": np.asarray(Wq)[cs, :].T.astype(f16),
            "wkt": np.asarray(Wk)[cs, :].T.astype(f16),
            "wvt": np.asarray(Wv)[cs, :].T.astype(f16),
            "wot": wot.astype(f16),
            "mt8": mt8[b],
            "bq_r": np.asarray(bq)[None, cs].astype(f16),
            "bk_r": np.asarray(bk)[None, cs].astype(f16),
            "bv_r": np.asarray(bv)[None, cs].astype(f16),
            "ones128": ones128, "ones512": ones512, "i8": i8,
        })
    return in_maps


def kernel(v, k, q, mask, Wq, bq, Wk, bk, Wv, bv, Wo, bo, _trace=False):
    from concourse.bass_utils import run_bass_kernel_spmd

    if "nc" not in _cache:
        _cache["nc"] = _build()
    nc = _cache["nc"]

    in_maps = _prep_inputs(v, k, q, mask, Wq, bq, Wk, bk, Wv, bv, Wo, bo)
    kw = {}
    if _trace:
        kw = dict(trace=True)
    res = run_bass_kernel_spmd(nc, in_maps, core_ids=list(range(NCORES)), **kw)
    _cache["last_result"] = res

    x = np.zeros((B, S, D), np.float32)
    attn = np.empty((B, H, S, S), np.float32)
    for c in range(NCORES):
        b, g = c % B, c // B
        out = res.results[c]
        x[b] += out["x_part"]
        at = out["attn_t"]
        for j in range(HPC):
            attn[b, g * HPC + j] = at[j].T
    x += np.asarray(bo, np.float32)[None, None, :]
    return x, attn


# revision 7
# speedup vs baseline: 1.5240x; 1.1764x over previous
"""TRN2 Bass kernel for nn_MultiHeadAttention (B=2, S=2048, D=1024, H=16).

Sharding: 8 cores = 2 batches x 4 head-groups (4 heads each).
Device math per core (transposed-P layout, fp16 matmuls, fp32 accumulation):
  qhT/khT = (Wslice @ xT + b)      [dc, s]   fp16, SBUF-resident
  vh      = (x @ WvT + bv)         [s, dc]   fp16 in SBUF with ones column
  P^T[k,q] = exp((K Q^T)/8 - 192*mask^T)     mask added in PSUM via fp8 matmul
  ctx^T,s  = [vh|1]^T @ P^T                  AV + rowsum in one matmul
  attn^T   = P^T * (1/s)                     DVE 2x pass, fp16 out to HBM
  x_part   = (ctx^T/s)^T @ WoT               host sums 4 partials per batch
"""

import sys

sys.path.insert(0, "/opt/trn_rl_repo")

import numpy as np
import ml_dtypes
from contextlib import ExitStack

B, S, D, H = 2, 2048, 1024, 16
HD = 64
HPC = 4            # heads per core
DC = HPC * HD      # 256 local channels per core
NCORES = 8
QC = 1024          # q-chunk (columns per attention inner tile)
MASKVAL = -192.0   # fp8-exact; exp(0.125*-192) = e^-24 ~ 3.8e-11

_cache = {}


def _build(s=S):
    """Build + compile the per-core Bass program (same for all cores)."""
    import concourse.tile as tile
    from concourse import bacc, mybir

    F32 = mybir.dt.float32
    F16 = mybir.dt.float16
    FP8 = mybir.dt.float8e4
    MM = mybir.AluOpType.mult
    EXPF = mybir.ActivationFunctionType.Exp

    qcw = min(QC, s)  # effective q-chunk
    nkb = s // 128    # k blocks
    nqc = s // qcw    # q chunks
    nsc = s // 128    # s chunks (output rows)
    nnq = s // 512    # projection column chunks
    nkc = D // 128    # contraction chunks

    nc = bacc.Bacc("TRN2", target_bir_lowering=False, debug=False,
                   num_devices=NCORES)

    XQT = nc.dram_tensor("xqt", [D, s], F16, kind="ExternalInput").ap()
    XKT = nc.dram_tensor("xkt", [D, s], F16, kind="ExternalInput").ap()
    XVT = nc.dram_tensor("xvt", [D, s], F16, kind="ExternalInput").ap()
    WQT = nc.dram_tensor("wqt", [D, DC], F16, kind="ExternalInput").ap()
    WKT = nc.dram_tensor("wkt", [D, DC], F16, kind="ExternalInput").ap()
    WVT = nc.dram_tensor("wvt", [D, DC], F16, kind="ExternalInput").ap()
    WOT = nc.dram_tensor("wot", [HD, HPC, D], F16, kind="ExternalInput").ap()
    MT8 = nc.dram_tensor("mt8", [s, s], FP8, kind="ExternalInput").ap()
    BQ = nc.dram_tensor("bq_r", [1, DC], F16, kind="ExternalInput").ap()
    BK = nc.dram_tensor("bk_r", [1, DC], F16, kind="ExternalInput").ap()
    BV = nc.dram_tensor("bv_r", [1, DC], F16, kind="ExternalInput").ap()
    ONES128 = nc.dram_tensor("ones128", [1, 128], F16, kind="ExternalInput").ap()
    ONES512 = nc.dram_tensor("ones512", [1, 512], F16, kind="ExternalInput").ap()
    I8 = nc.dram_tensor("i8", [128, 128], FP8, kind="ExternalInput").ap()

    ATT = nc.dram_tensor("attn_t", [HPC, s, s], F16, kind="ExternalOutput").ap()
    XO = nc.dram_tensor("x_part", [s, D], F32, kind="ExternalOutput").ap()

    with tile.TileContext(nc) as tc, ExitStack() as ctx:
        # ---- long-lived pools ----
        persist = ctx.enter_context(tc.tile_pool(name="persist", bufs=1))

        mask_sb = persist.tile([128, nkb, s], FP8, tag="mask")
        for kb in range(nkb):
            nc.sync.dma_start(mask_sb[:, kb, :], MT8[kb * 128:(kb + 1) * 128, :])
        i8_sb = persist.tile([128, 128], FP8, tag="i8")
        nc.sync.dma_start(i8_sb[:], I8)
        consts = persist.tile([1, 128 + 512 + 3 * DC], F16, tag="consts")
        ones128_sb = consts[:, 0:128]
        ones512_sb = consts[:, 128:640]
        bq_sb = consts[:, 640:640 + DC]
        bk_sb = consts[:, 640 + DC:640 + 2 * DC]
        bv_sb = consts[:, 640 + 2 * DC:640 + 3 * DC]
        nc.sync.dma_start(ones128_sb, ONES128)
        nc.sync.dma_start(ones512_sb, ONES512)
        nc.sync.dma_start(bq_sb, BQ)
        nc.sync.dma_start(bk_sb, BK)
        nc.sync.dma_start(bv_sb, BV)
        woT_sb = persist.tile([HD, HPC, D], F16, tag="wot")
        nc.sync.dma_start(woT_sb[:], WOT)

        # vh + interleaved ones column: [128, sc, (h,65)]
        vh_sb = persist.tile([128, nsc, HPC * 65], F16, tag="vh")
        ones_view = vh_sb[:, :, :].rearrange(
            "p c (h d) -> p c h d", h=HPC, d=65)[:, :, :, 64:65]
        nc.gpsimd.memset(ones_view, 1.0)

        qhT_sb = persist.tile([128, 2, s], F16, tag="qhT")
        khT_sb = persist.tile([128, 2, s], F16, tag="khT")
        ctxT_sb = persist.tile([HD, HPC, s], F16, tag="ctxT")

        # ================= Phase 1: projections =================
        with tc.tile_pool(name="p1w", bufs=1) as p1w, \
             tc.tile_pool(name="p1x", bufs=6) as p1x, \
             tc.tile_pool(name="p1ps", bufs=1, space="PSUM") as p1ps:
            wq_sb = p1w.tile([128, nkc, DC], F16, tag="wq")
            nc.sync.dma_start(wq_sb[:], WQT.rearrange("(c p) n -> p c n", p=128))
            wk_sb = p1w.tile([128, nkc, DC], F16, tag="wk")
            nc.sync.dma_start(wk_sb[:], WKT.rearrange("(c p) n -> p c n", p=128))
            wv_sb = p1w.tile([128, nkc, DC], F16, tag="wv")
            nc.sync.dma_start(wv_sb[:], WVT.rearrange("(c p) n -> p c n", p=128))
            for nq in range(nnq):
                ncol = slice(nq * 512, (nq + 1) * 512)
                psq = [p1ps.tile([128, 512], F32, tag=f"psq{m}", name=f"psq{m}")
                       for m in range(2)]
                psk = [p1ps.tile([128, 512], F32, tag=f"psk{m}", name=f"psk{m}")
                       for m in range(2)]
                psv = [p1ps.tile([128, 256], F32, tag=f"psv{j}", name=f"psv{j}")
                       for j in range(4)]
                # bias seeds (K=1)
                for m in range(2):
                    mm = slice(m * 128, (m + 1) * 128)
                    nc.tensor.matmul(psq[m][:], bq_sb[0:1, mm], ones512_sb[0:1, :],
                                     start=True, stop=False)
                    nc.tensor.matmul(psk[m][:], bk_sb[0:1, mm], ones512_sb[0:1, :],
                                     start=True, stop=False)
                for j in range(4):
                    nc.tensor.matmul(psv[j][:], ones128_sb[0:1, 0:128],
                                     bv_sb[0:1, 0:256], start=True, stop=False)
                for kc in range(nkc):
                    last = kc == nkc - 1
                    xq_c = p1x.tile([128, 512], F16, tag="x")
                    nc.sync.dma_start(xq_c[:], XQT[kc * 128:(kc + 1) * 128, ncol])
                    xk_c = p1x.tile([128, 512], F16, tag="x")
                    nc.sync.dma_start(xk_c[:], XKT[kc * 128:(kc + 1) * 128, ncol])
                    xv_c = p1x.tile([128, 512], F16, tag="x")
                    nc.sync.dma_start(xv_c[:], XVT[kc * 128:(kc + 1) * 128, ncol])
                    for m in range(2):
                        mm = slice(m * 128, (m + 1) * 128)
                        nc.tensor.matmul(psq[m][:], wq_sb[:, kc, mm], xq_c[:],
                                         start=False, stop=last)
                        nc.tensor.matmul(psk[m][:], wk_sb[:, kc, mm], xk_c[:],
                                         start=False, stop=last)
                    for sl in range(4):
                        nc.tensor.matmul(psv[sl][:],
                                         xv_c[:, sl * 128:(sl + 1) * 128],
                                         wv_sb[:, kc, :],
                                         start=False, stop=last)
                # drain psums into fp16 SBUF residents
                with nc.allow_low_precision(reason="fp16 proj"):
                    for m in range(2):
                        nc.vector.tensor_copy(qhT_sb[:, m, ncol], psq[m][:])
                        nc.vector.tensor_copy(khT_sb[:, m, ncol], psk[m][:])
                    for sl in range(4):
                        sc = nq * 4 + sl
                        dst = vh_sb[:, sc:sc + 1, :].rearrange(
                            "p one (h d) -> p (one h) d", h=HPC, d=65)[:, :, 0:64]
                        src = psv[sl][:].rearrange("p (h d) -> p h d", h=HPC, d=64)
                        nc.vector.tensor_copy(dst, src)

        # ================= Phase 2: attention =================
        with tc.tile_pool(name="pP", bufs=6) as pP, \
             tc.tile_pool(name="prow", bufs=3) as prow, \
             tc.tile_pool(name="prep", bufs=2) as prep, \
             tc.tile_pool(name="prep32", bufs=2) as prep32, \
             tc.tile_pool(name="pxo", bufs=2) as pxo, \
             tc.tile_pool(name="ppl", bufs=3, space="PSUM") as ppl, \
             tc.tile_pool(name="pctx", bufs=1, space="PSUM") as pctx:

            for h in range(HPC):
                hp = slice((h % 2) * 64, (h % 2) * 64 + 64)
                hm = h // 2
                for qc in range(nqc):
                    qsl = slice(qc * qcw, (qc + 1) * qcw)
                    Ph = [pP.tile([128, 4, qcw], F16, tag="P", name=f"P{i}")
                          for i in range(nkb // 4)]
                    cps = pctx.tile([65, qcw], F32, tag="ctx")
                    # -- phase A: QK^T + mask + exp (PE never waits on ACT) --
                    for kb in range(nkb):
                        pl = ppl.tile([128, qcw], F32, tag="pl")
                        for u in range(qcw // 512):
                            us = slice(u * 512, (u + 1) * 512)
                            qus = slice(qc * qcw + u * 512, qc * qcw + u * 512 + 512)
                            nc.tensor.matmul(pl[:, us],
                                             khT_sb[hp, hm, kb * 128:(kb + 1) * 128],
                                             qhT_sb[hp, hm, qus],
                                             start=True, stop=False)
                            nc.tensor.matmul(pl[:, us], i8_sb[:],
                                             mask_sb[:, kb, qus],
                                             start=False, stop=True)
                        pslice = Ph[kb // 4][:, kb % 4, :]
                        nc.scalar.activation(pslice, pl[:], EXPF, scale=0.125)
                    # -- phase B: AV + rowsum accumulation --
                    for kb in range(nkb):
                        pslice = Ph[kb // 4][:, kb % 4, :]
                        for u in range(qcw // 512):
                            us = slice(u * 512, (u + 1) * 512)
                            nc.tensor.matmul(cps[:, us],
                                             vh_sb[:, kb, h * 65:(h + 1) * 65],
                                             pslice[:, us],
                                             start=(kb == 0),
                                             stop=(kb == nkb - 1))
                    # -- softmax denominators: replicate s then fast recip --
                    s_sb = prow.tile([1, qcw], F16, tag="row")
                    nc.scalar.copy(s_sb[:], cps[64:65, :])
                    rep_ps = ppl.tile([128, qcw], F32, tag="pl", name="rep_ps")
                    for u in range(qcw // 512):
                        us = slice(u * 512, (u + 1) * 512)
                        nc.tensor.matmul(rep_ps[:, us], ones128_sb[0:1, :],
                                         s_sb[0:1, us], start=True, stop=True)
                    rep32 = prep32.tile([128, qcw], F32, tag="rep32")
                    nc.vector.reciprocal_approx_fast(rep32[:], rep_ps[:])
                    rep_sb = prep.tile([128, qcw], F16, tag="repsb")
                    with nc.allow_low_precision(reason="fp16 rep"):
                        nc.vector.tensor_copy(rep_sb[:], rep32[:])
                    # normalized ctx^T slice (fp16 out)
                    with nc.allow_low_precision(reason="fp16 ctx"):
                        nc.vector.tensor_tensor(ctxT_sb[0:HD, h, qsl],
                                                cps[0:HD, :], rep_sb[0:HD, :], MM)
                    # normalize P in place (fp16 2x), batched attn^T writes
                    att_t = ATT[h].rearrange("(kb p) q -> p kb q", p=128)
                    for kb in range(nkb):
                        pslice = Ph[kb // 4][:, kb % 4, :]
                        with nc.allow_low_precision(reason="fp16 attn"):
                            nc.vector.tensor_tensor(pslice, pslice,
                                                    rep_sb[:], MM)
                    for i in range(nkb // 4):
                        nc.sync.dma_start(att_t[:, 4 * i:4 * i + 4, qsl], Ph[i][:])

            # ================= Phase 3: output projection =================
            for sc in range(nsc):
                ssl = slice(sc * 128, (sc + 1) * 128)
                xo = pxo.tile([128, D], F32, tag="xo")
                for nh in range(2):
                    px = ppl.tile([128, qcw], F32, tag="pl", name="px")[:, 0:512]
                    for h in range(HPC):
                        nc.tensor.matmul(px[:], ctxT_sb[0:HD, h, ssl],
                                         woT_sb[0:HD, h, nh * 512:(nh + 1) * 512],
                                         start=(h == 0), stop=(h == HPC - 1))
                    nc.vector.tensor_copy(xo[:, nh * 512:(nh + 1) * 512], px[:])
                nc.sync.dma_start(XO[ssl, :], xo[:])

    nc.compile()
    return nc


def _prep_inputs(v, k, q, mask, Wq, bq, Wk, bk, Wv, bv, Wo, bo):
    """Host-side shard prep. Returns per-core input maps."""
    f16 = np.float16
    ones128 = np.ones((1, 128), f16)
    ones512 = np.ones((1, 512), f16)
    i8 = np.eye(128, dtype=np.float32).astype(ml_dtypes.float8_e4m3)

    xT = {}
    mt8 = {}
    for b in range(B):
        xT[("q", b)] = np.ascontiguousarray(np.asarray(q[b]).T).astype(f16)
        xT[("k", b)] = np.ascontiguousarray(np.asarray(k[b]).T).astype(f16)
        xT[("v", b)] = np.ascontiguousarray(np.asarray(v[b]).T).astype(f16)
        mt8[b] = np.ascontiguousarray(
            np.asarray(mask[b, 0], np.float32).T * MASKVAL
        ).astype(ml_dtypes.float8_e4m3)

    in_maps = []
    for c in range(NCORES):
        b, g = c % B, c // B
        cs = slice(g * DC, (g + 1) * DC)
        wot = np.ascontiguousarray(
            np.asarray(Wo)[:, cs].T.reshape(HPC, HD, D).transpose(1, 0, 2))
        in_maps.append({
            "xqt": xT[("q", b)], "xkt": xT[("k", b)], "xvt": xT[("v", b)],
            "wqt": np.asarray(Wq)[cs, :].T.astype(f16),
            "wkt": np.asarray(Wk)[cs, :].T.astype(f16),
            "wvt": np.asarray(Wv)[cs, :].T.astype(f16),
            "wot": wot.astype(f16),
            "mt8": mt8[b],
            "bq_r": np.asarray(bq)[None, cs].astype(f16),
            "bk_r": np.asarray(bk)[None, cs].astype(f16),
            "bv_r": np.asarray(bv)[None, cs].astype(f16),
            "ones128": ones128, "ones512": ones512, "i8": i8,
        })
    return in_maps


def kernel(v, k, q, mask, Wq, bq, Wk, bk, Wv, bv, Wo, bo, _trace=False):
    from concourse.bass_utils import run_bass_kernel_spmd

    if "nc" not in _cache:
        _cache["nc"] = _build()
    nc = _cache["nc"]

    in_maps = _prep_inputs(v, k, q, mask, Wq, bq, Wk, bk, Wv, bv, Wo, bo)
    kw = {}
    if _trace:
        kw = dict(trace=True)
    res = run_bass_kernel_spmd(nc, in_maps, core_ids=list(range(NCORES)), **kw)
    _cache["last_result"] = res

    x = np.zeros((B, S, D), np.float32)
    attn = np.empty((B, H, S, S), np.float32)
    for c in range(NCORES):
        b, g = c % B, c // B
        out = res.results[c]
        x[b] += out["x_part"]
        at = out["attn_t"]
        for j in range(HPC):
            attn[b, g * HPC + j] = at[j].T
    x += np.asarray(bo, np.float32)[None, None, :]
    return x, attn


# revision 8
# speedup vs baseline: 1.7525x; 1.1500x over previous
"""TRN2 Bass kernel for nn_MultiHeadAttention (B=2, S=2048, D=1024, H=16).

Sharding: 8 cores = 2 batches x 4 head-groups (4 heads each).
Device math per core (transposed-P layout, fp16 matmuls, fp32 accumulation):
  qhT/khT = (Wslice @ xT + b)      [dc, s]   fp16, SBUF-resident
  vh      = (x @ WvT + bv)         [s, dc]   fp16 in SBUF with ones column
  P^T[k,q] = exp((K Q^T)/8 - 192*mask^T)     mask added in PSUM via fp8 matmul
  ctx^T,s  = [vh|1]^T @ P^T                  AV + rowsum in one matmul
  attn^T   = P^T * (1/s)                     DVE 2x pass, fp16 out to HBM
  x_part   = (ctx^T/s)^T @ WoT               host sums 4 partials per batch
"""

import sys

sys.path.insert(0, "/opt/trn_rl_repo")

import numpy as np
import ml_dtypes
from contextlib import ExitStack

B, S, D, H = 2, 2048, 1024, 16
HD = 64
HPC = 4            # heads per core
DC = HPC * HD      # 256 local channels per core
NCORES = 8
QC = 1024          # q-chunk (columns per attention inner tile)
MASKVAL = -192.0   # fp8-exact; exp(0.125*-192) = e^-24 ~ 3.8e-11

_cache = {}


def _build(s=S):
    """Build + compile the per-core Bass program (same for all cores)."""
    import concourse.tile as tile
    from concourse import bacc, mybir

    F32 = mybir.dt.float32
    F16 = mybir.dt.float16
    FP8 = mybir.dt.float8e4
    MM = mybir.AluOpType.mult
    EXPF = mybir.ActivationFunctionType.Exp

    qcw = min(QC, s)  # effective q-chunk
    nkb = s // 128    # k blocks
    nqc = s // qcw    # q chunks
    nsc = s // 128    # s chunks (output rows)
    nnq = s // 512    # projection column chunks
    nkc = D // 128    # contraction chunks

    nc = bacc.Bacc("TRN2", target_bir_lowering=False, debug=False,
                   num_devices=NCORES)

    XQT = nc.dram_tensor("xqt", [D, s], F16, kind="ExternalInput").ap()
    XKT = nc.dram_tensor("xkt", [D, s], F16, kind="ExternalInput").ap()
    XVT = nc.dram_tensor("xvt", [D, s], F16, kind="ExternalInput").ap()
    WQT = nc.dram_tensor("wqt", [D, DC], F16, kind="ExternalInput").ap()
    WKT = nc.dram_tensor("wkt", [D, DC], F16, kind="ExternalInput").ap()
    WVT = nc.dram_tensor("wvt", [D, DC], F16, kind="ExternalInput").ap()
    WOT = nc.dram_tensor("wot", [HD, HPC, D], F16, kind="ExternalInput").ap()
    MT8 = nc.dram_tensor("mt8", [s, s], FP8, kind="ExternalInput").ap()
    BQ = nc.dram_tensor("bq_r", [1, DC], F16, kind="ExternalInput").ap()
    BK = nc.dram_tensor("bk_r", [1, DC], F16, kind="ExternalInput").ap()
    BV = nc.dram_tensor("bv_r", [1, DC], F16, kind="ExternalInput").ap()
    ONES128 = nc.dram_tensor("ones128", [1, 128], F16, kind="ExternalInput").ap()
    ONES512 = nc.dram_tensor("ones512", [1, 512], F16, kind="ExternalInput").ap()
    I8 = nc.dram_tensor("i8", [128, 128], FP8, kind="ExternalInput").ap()

    ATT = nc.dram_tensor("attn_t", [HPC, s, s], F16, kind="ExternalOutput").ap()
    XO = nc.dram_tensor("x_part", [s, D], F32, kind="ExternalOutput").ap()

    with tile.TileContext(nc) as tc, ExitStack() as ctx:
        # ---- long-lived pools ----
        persist = ctx.enter_context(tc.tile_pool(name="persist", bufs=1))

        mask_sb = persist.tile([128, nkb, s], FP8, tag="mask")
        for kb in range(nkb):
            nc.sync.dma_start(mask_sb[:, kb, :], MT8[kb * 128:(kb + 1) * 128, :])
        i8_sb = persist.tile([128, 128], FP8, tag="i8")
        nc.sync.dma_start(i8_sb[:], I8)
        consts = persist.tile([1, 128 + 512 + 3 * DC], F16, tag="consts")
        ones128_sb = consts[:, 0:128]
        ones512_sb = consts[:, 128:640]
        bq_sb = consts[:, 640:640 + DC]
        bk_sb = consts[:, 640 + DC:640 + 2 * DC]
        bv_sb = consts[:, 640 + 2 * DC:640 + 3 * DC]
        nc.sync.dma_start(ones128_sb, ONES128)
        nc.sync.dma_start(ones512_sb, ONES512)
        nc.sync.dma_start(bq_sb, BQ)
        nc.sync.dma_start(bk_sb, BK)
        nc.sync.dma_start(bv_sb, BV)
        woT_sb = persist.tile([HD, HPC, D], F16, tag="wot")
        nc.sync.dma_start(woT_sb[:], WOT)

        # vh + interleaved ones column: [128, sc, (h,65)]
        vh_sb = persist.tile([128, nsc, HPC * 65], F16, tag="vh")
        ones_view = vh_sb[:, :, :].rearrange(
            "p c (h d) -> p c h d", h=HPC, d=65)[:, :, :, 64:65]
        nc.gpsimd.memset(ones_view, 1.0)

        qhT_sb = persist.tile([128, 2, s], F16, tag="qhT")
        khT_sb = persist.tile([128, 2, s], F16, tag="khT")
        ctxT_sb = persist.tile([HD, HPC, s], F16, tag="ctxT")

        # ================= Phase 1: projections =================
        with tc.tile_pool(name="p1w", bufs=1) as p1w, \
             tc.tile_pool(name="p1x", bufs=6) as p1x, \
             tc.tile_pool(name="p1ps", bufs=1, space="PSUM") as p1ps:
            wq_sb = p1w.tile([128, nkc, DC], F16, tag="wq")
            nc.sync.dma_start(wq_sb[:], WQT.rearrange("(c p) n -> p c n", p=128))
            wk_sb = p1w.tile([128, nkc, DC], F16, tag="wk")
            nc.sync.dma_start(wk_sb[:], WKT.rearrange("(c p) n -> p c n", p=128))
            wv_sb = p1w.tile([128, nkc, DC], F16, tag="wv")
            nc.sync.dma_start(wv_sb[:], WVT.rearrange("(c p) n -> p c n", p=128))
            for nq in range(nnq):
                ncol = slice(nq * 512, (nq + 1) * 512)
                psq = [p1ps.tile([128, 512], F32, tag=f"psq{m}", name=f"psq{m}")
                       for m in range(2)]
                psk = [p1ps.tile([128, 512], F32, tag=f"psk{m}", name=f"psk{m}")
                       for m in range(2)]
                psv = [p1ps.tile([128, 256], F32, tag=f"psv{j}", name=f"psv{j}")
                       for j in range(4)]
                # bias seeds (K=1)
                for m in range(2):
                    mm = slice(m * 128, (m + 1) * 128)
                    nc.tensor.matmul(psq[m][:], bq_sb[0:1, mm], ones512_sb[0:1, :],
                                     start=True, stop=False)
                    nc.tensor.matmul(psk[m][:], bk_sb[0:1, mm], ones512_sb[0:1, :],
                                     start=True, stop=False)
                for j in range(4):
                    nc.tensor.matmul(psv[j][:], ones128_sb[0:1, 0:128],
                                     bv_sb[0:1, 0:256], start=True, stop=False)
                for kc in range(nkc):
                    last = kc == nkc - 1
                    xq_c = p1x.tile([128, 512], F16, tag="x")
                    nc.sync.dma_start(xq_c[:], XQT[kc * 128:(kc + 1) * 128, ncol])
                    xk_c = p1x.tile([128, 512], F16, tag="x")
                    nc.sync.dma_start(xk_c[:], XKT[kc * 128:(kc + 1) * 128, ncol])
                    xv_c = p1x.tile([128, 512], F16, tag="x")
                    nc.sync.dma_start(xv_c[:], XVT[kc * 128:(kc + 1) * 128, ncol])
                    for m in range(2):
                        mm = slice(m * 128, (m + 1) * 128)
                        nc.tensor.matmul(psq[m][:], wq_sb[:, kc, mm], xq_c[:],
                                         start=False, stop=last)
                        nc.tensor.matmul(psk[m][:], wk_sb[:, kc, mm], xk_c[:],
                                         start=False, stop=last)
                    for sl in range(4):
                        nc.tensor.matmul(psv[sl][:],
                                         xv_c[:, sl * 128:(sl + 1) * 128],
                                         wv_sb[:, kc, :],
                                         start=False, stop=last)
                # drain psums into fp16 SBUF residents
                with nc.allow_low_precision(reason="fp16 proj"):
                    for m in range(2):
                        nc.vector.tensor_copy(qhT_sb[:, m, ncol], psq[m][:])
                        nc.vector.tensor_copy(khT_sb[:, m, ncol], psk[m][:])
                    for sl in range(4):
                        sc = nq * 4 + sl
                        dst = vh_sb[:, sc:sc + 1, :].rearrange(
                            "p one (h d) -> p (one h) d", h=HPC, d=65)[:, :, 0:64]
                        src = psv[sl][:].rearrange("p (h d) -> p h d", h=HPC, d=64)
                        nc.vector.tensor_copy(dst, src)

        # ================= Phase 2: attention =================
        with tc.tile_pool(name="pP", bufs=8) as pP, \
             tc.tile_pool(name="prow", bufs=3) as prow, \
             tc.tile_pool(name="prep", bufs=2) as prep, \
             tc.tile_pool(name="prep32", bufs=2) as prep32, \
             tc.tile_pool(name="pxo", bufs=2) as pxo, \
             tc.tile_pool(name="ppl", bufs=3, space="PSUM") as ppl, \
             tc.tile_pool(name="pctx", bufs=1, space="PSUM") as pctx:

            for qc in range(nqc):
                qsl = slice(qc * qcw, (qc + 1) * qcw)
                for h in range(HPC):
                    hp = slice((h % 2) * 64, (h % 2) * 64 + 64)
                    hm = h // 2
                    Ph = [pP.tile([128, 4, qcw], F16, tag="P", name=f"P{i}")
                          for i in range(nkb // 4)]
                    cps = pctx.tile([65, qcw], F32, tag="ctx")
                    # -- phase A: QK^T + mask + exp (PE never waits on ACT) --
                    for kb in range(nkb):
                        pl = ppl.tile([128, qcw], F32, tag="pl")
                        for u in range(qcw // 512):
                            us = slice(u * 512, (u + 1) * 512)
                            qus = slice(qc * qcw + u * 512, qc * qcw + u * 512 + 512)
                            nc.tensor.matmul(pl[:, us],
                                             khT_sb[hp, hm, kb * 128:(kb + 1) * 128],
                                             qhT_sb[hp, hm, qus],
                                             start=True, stop=False)
                        for u in range(qcw // 512):
                            us = slice(u * 512, (u + 1) * 512)
                            qus = slice(qc * qcw + u * 512, qc * qcw + u * 512 + 512)
                            nc.tensor.matmul(pl[:, us], i8_sb[:],
                                             mask_sb[:, kb, qus],
                                             start=False, stop=True)
                        pslice = Ph[kb // 4][:, kb % 4, :]
                        nc.scalar.activation(pslice, pl[:], EXPF, scale=0.125)
                    # -- phase B: AV + rowsum accumulation --
                    for kb in range(nkb):
                        pslice = Ph[kb // 4][:, kb % 4, :]
                        for u in range(qcw // 512):
                            us = slice(u * 512, (u + 1) * 512)
                            nc.tensor.matmul(cps[:, us],
                                             vh_sb[:, kb, h * 65:(h + 1) * 65],
                                             pslice[:, us],
                                             start=(kb == 0),
                                             stop=(kb == nkb - 1))
                    # -- softmax denominators: replicate s then fast recip --
                    s_sb = prow.tile([1, qcw], F16, tag="row")
                    nc.scalar.copy(s_sb[:], cps[64:65, :])
                    rep_ps = ppl.tile([128, qcw], F32, tag="pl", name="rep_ps")
                    for u in range(qcw // 512):
                        us = slice(u * 512, (u + 1) * 512)
                        nc.tensor.matmul(rep_ps[:, us], ones128_sb[0:1, :],
                                         s_sb[0:1, us], start=True, stop=True)
                    rep32 = prep32.tile([128, qcw], F32, tag="rep32")
                    nc.vector.reciprocal_approx_fast(rep32[:], rep_ps[:])
                    rep_sb = prep.tile([128, qcw], F16, tag="repsb")
                    with nc.allow_low_precision(reason="fp16 rep"):
                        nc.vector.tensor_copy(rep_sb[:], rep32[:])
                    # normalized ctx^T slice (fp16 out)
                    with nc.allow_low_precision(reason="fp16 ctx"):
                        nc.vector.tensor_tensor(ctxT_sb[0:HD, h, qsl],
                                                cps[0:HD, :], rep_sb[0:HD, :], MM)
                    # normalize P in place (fp16 2x), batched attn^T writes
                    att_t = ATT[h].rearrange("(kb p) q -> p kb q", p=128)
                    for kb in range(nkb):
                        pslice = Ph[kb // 4][:, kb % 4, :]
                        with nc.allow_low_precision(reason="fp16 attn"):
                            nc.vector.tensor_tensor(pslice, pslice,
                                                    rep_sb[:], MM)
                    for i in range(nkb // 4):
                        nc.sync.dma_start(att_t[:, 4 * i:4 * i + 4, qsl], Ph[i][:])

                # ---- output projection for this qc's s-rows ----
                for sc in range(qc * (qcw // 128), (qc + 1) * (qcw // 128)):
                    ssl = slice(sc * 128, (sc + 1) * 128)
                    xo = pxo.tile([128, D], F32, tag="xo")
                    for nh in range(2):
                        px = ppl.tile([128, qcw], F32, tag="pl", name="px")[:, 0:512]
                        for h in range(HPC):
                            nc.tensor.matmul(px[:], ctxT_sb[0:HD, h, ssl],
                                             woT_sb[0:HD, h,
                                                    nh * 512:(nh + 1) * 512],
                                             start=(h == 0), stop=(h == HPC - 1))
                        nc.vector.tensor_copy(xo[:, nh * 512:(nh + 1) * 512], px[:])
                    nc.sync.dma_start(XO[ssl, :], xo[:])

    nc.compile()
    return nc


def _prep_inputs(v, k, q, mask, Wq, bq, Wk, bk, Wv, bv, Wo, bo):
    """Host-side shard prep. Returns per-core input maps."""
    f16 = np.float16
    ones128 = np.ones((1, 128), f16)
    ones512 = np.ones((1, 512), f16)
    i8 = np.eye(128, dtype=np.float32).astype(ml_dtypes.float8_e4m3)

    xT = {}
    mt8 = {}
    for b in range(B):
        xT[("q", b)] = np.ascontiguousarray(np.asarray(q[b]).T).astype(f16)
        xT[("k", b)] = np.ascontiguousarray(np.asarray(k[b]).T).astype(f16)
        xT[("v", b)] = np.ascontiguousarray(np.asarray(v[b]).T).astype(f16)
        mt8[b] = np.ascontiguousarray(
            np.asarray(mask[b, 0], np.float32).T * MASKVAL
        ).astype(ml_dtypes.float8_e4m3)

    in_maps = []
    for c in range(NCORES):
        b, g = c % B, c // B
        cs = slice(g * DC, (g + 1) * DC)
        wot = np.ascontiguousarray(
            np.asarray(Wo)[:, cs].T.reshape(HPC, HD, D).transpose(1, 0, 2))
        in_maps.append({
            "xqt": xT[("q", b)], "xkt": xT[("k", b)], "xvt": xT[("v", b)],
            "wqt": np.asarray(Wq)[cs, :].T.astype(f16),
            "wkt": np.asarray(Wk)[cs, :].T.astype(f16),
            "wvt": np.asarray(Wv)[cs, :].T.astype(f16),
            "wot": wot.astype(f16),
            "mt8": mt8[b],
            "bq_r": np.asarray(bq)[None, cs].astype(f16),
            "bk_r": np.asarray(bk)[None, cs].astype(f16),
            "bv_r": np.asarray(bv)[None, cs].astype(f16),
            "ones128": ones128, "ones512": ones512, "i8": i8,
        })
    return in_maps


def kernel(v, k, q, mask, Wq, bq, Wk, bk, Wv, bv, Wo, bo, _trace=False):
    from concourse.bass_utils import run_bass_kernel_spmd

    if "nc" not in _cache:
        _cache["nc"] = _build()
    nc = _cache["nc"]

    in_maps = _prep_inputs(v, k, q, mask, Wq, bq, Wk, bk, Wv, bv, Wo, bo)
    kw = {}
    if _trace:
        kw = dict(trace=True)
    res = run_bass_kernel_spmd(nc, in_maps, core_ids=list(range(NCORES)), **kw)
    _cache["last_result"] = res

    x = np.zeros((B, S, D), np.float32)
    attn = np.empty((B, H, S, S), np.float32)
    for c in range(NCORES):
        b, g = c % B, c // B
        out = res.results[c]
        x[b] += out["x_part"]
        at = out["attn_t"]
        for j in range(HPC):
            attn[b, g * HPC + j] = at[j].T
    x += np.asarray(bo, np.float32)[None, None, :]
    return x, attn
